# revision 1
# baseline (speedup 1.0000x reference)
"""AttentionBlock kernel for 8 Trainium2 NeuronCores.

Reference (per batch element b):
    q = x @ Wq.T + bq; k = x @ Wk.T + bk; v = x @ Wv.T + bv
    scores[q,s] = q . k, causal-masked (s <= q valid)
    probs = softmax(scores / sqrt(512), axis=QUERY)   # normalized over q!
    attn = probs @ v
    out = concat([x, attn], -1)

Sharding: data-parallel over batch B=8 -> one batch element per core,
weights replicated, no collectives.

Device layout trick: compute S_T = scores^T in [s, q] layout so the
query-axis softmax becomes a free-axis (per-partition) reduction, and
S_T is directly the lhsT operand for attn = S_T.T @ v. The
normalization (1/denom[s]) rides on v rows instead of on the big E
matrix. Host pre-transposes x and the weights so every matmul
contraction lands on the partition axis.
"""

import numpy as np

B, T, C, K = 8, 2048, 512, 512
P = 128
NCC = C // P  # contraction chunks (4)
ND = K // P  # dk chunks (4)
NT = T // P  # 16 row tiles
NQB = T // 512  # 4 query blocks of 512
SCALE = 1.0 / np.sqrt(512.0)
N_CORES = 8

_CACHE = {}


def _build_nc():
    from contextlib import ExitStack

    import concourse.bass as bass
    import concourse.tile as tile
    from concourse import bacc, mybir

    f16 = mybir.dt.float16
    f32 = mybir.dt.float32

    nc = bacc.Bacc("TRN2", target_bir_lowering=False, debug=False)

    xT_h = nc.dram_tensor("xT", [C, T], f16, kind="ExternalInput")
    wqT_h = nc.dram_tensor("wqT", [C, K], f16, kind="ExternalInput")
    wkT_h = nc.dram_tensor("wkT", [C, K], f16, kind="ExternalInput")
    wvT_h = nc.dram_tensor("wvT", [C, K], f16, kind="ExternalInput")
    bq_h = nc.dram_tensor("bq", [K], f32, kind="ExternalInput")
    bk_h = nc.dram_tensor("bk", [K], f32, kind="ExternalInput")
    bv_h = nc.dram_tensor("bv", [K], f16, kind="ExternalInput")
    attn_h = nc.dram_tensor("attn", [T, K], f32, kind="ExternalOutput")
    attn_d = attn_h.ap()

    with tile.TileContext(nc) as tc, ExitStack() as ctx:
        sb = ctx.enter_context(tc.tile_pool(name="sb", bufs=1))
        ao_pool = ctx.enter_context(tc.tile_pool(name="ao", bufs=4))
        ps_qkv = ctx.enter_context(tc.tile_pool(name="ps_qkv", bufs=2, space="PSUM"))
        ps_st = ctx.enter_context(tc.tile_pool(name="ps_st", bufs=4, space="PSUM"))
        ps_at = ctx.enter_context(tc.tile_pool(name="ps_at", bufs=2, space="PSUM"))

        # ---- constants / inputs to SBUF ----
        wq = [sb.tile([P, K], f16, name=f"wq{cc}", tag=f"wq{cc}") for cc in range(NCC)]
        wk = [sb.tile([P, K], f16, name=f"wk{cc}", tag=f"wk{cc}") for cc in range(NCC)]
        wv = [sb.tile([P, K], f16, name=f"wv{cc}", tag=f"wv{cc}") for cc in range(NCC)]
        for cc in range(NCC):
            nc.sync.dma_start(wq[cc][:], wqT_h.ap()[cc * P : (cc + 1) * P, :])
            nc.sync.dma_start(wk[cc][:], wkT_h.ap()[cc * P : (cc + 1) * P, :])
            nc.sync.dma_start(wv[cc][:], wvT_h.ap()[cc * P : (cc + 1) * P, :])

        # biases: bq/bk as [p, chunk] (per-partition scalars for the dk chunk),
        # bv broadcast across partitions (it adds along the free axis of v)
        bq_sb = sb.tile([P, ND], f32, tag="bq_sb")
        bk_sb = sb.tile([P, ND], f32, tag="bk_sb")
        nc.sync.dma_start(
            bq_sb[:], bass.AP(tensor=bq_h, offset=0, ap=[[1, P], [P, ND]])
        )
        nc.sync.dma_start(
            bk_sb[:], bass.AP(tensor=bk_h, offset=0, ap=[[1, P], [P, ND]])
        )
        bv_sb = sb.tile([P, K], f16, tag="bv_sb")
        nc.sync.dma_start(
            bv_sb[:], bass.AP(tensor=bv_h, offset=0, ap=[[0, P], [1, K]])
        )

        # x^T in [c, t] layout, 4 partition chunks x 4 column blocks
        xts = [
            [
                sb.tile([P, 512], f16, name=f"xts{cc}_{tb}", tag=f"xts{cc}_{tb}")
                for tb in range(NQB)
            ]
            for cc in range(NCC)
        ]
        for cc in range(NCC):
            for tb in range(NQB):
                nc.sync.dma_start(
                    xts[cc][tb][:],
                    xT_h.ap()[cc * P : (cc + 1) * P, tb * 512 : (tb + 1) * 512],
                )

        # ---- persistent intermediates ----
        qt = [
            [sb.tile([P, 512], f16, name=f"qt{d}_{qb}", tag=f"qt{d}_{qb}") for qb in range(NQB)]
            for d in range(ND)
        ]
        kt = [
            [sb.tile([P, 512], f16, name=f"kt{d}_{tb}", tag=f"kt{d}_{tb}") for tb in range(NQB)]
            for d in range(ND)
        ]
        v_s = [sb.tile([P, K], f16, name=f"v{ti}", tag=f"v{ti}") for ti in range(NT)]
        E = [sb.tile([P, T], f16, name=f"E{si}", tag=f"E{si}") for si in range(NT)]
        den = [sb.tile([P, 1], f32, name=f"den{si}", tag=f"den{si}") for si in range(NT)]
        rden = [sb.tile([P, 1], f32, name=f"rden{si}", tag=f"rden{si}") for si in range(NT)]

        Ident = mybir.ActivationFunctionType.Identity
        Exp = mybir.ActivationFunctionType.Exp
        Copy = mybir.ActivationFunctionType.Copy

        # ---- phase 1: qT/kT in [dk, t] layout, v in [t, dv] layout ----
        for d in range(ND):
            for qb in range(NQB):
                ps = ps_qkv.tile([P, 512], f32)
                for cc in range(NCC):
                    nc.tensor.matmul(
                        ps[:],
                        lhsT=wq[cc][:, d * P : (d + 1) * P],
                        rhs=xts[cc][qb][:],
                        start=(cc == 0),
                        stop=(cc == NCC - 1),
                    )
                nc.scalar.activation(
                    qt[d][qb][:], ps[:], Ident, bias=bq_sb[:, d : d + 1], scale=1.0
                )
        for d in range(ND):
            for tb in range(NQB):
                ps = ps_qkv.tile([P, 512], f32)
                for cc in range(NCC):
                    nc.tensor.matmul(
                        ps[:],
                        lhsT=wk[cc][:, d * P : (d + 1) * P],
                        rhs=xts[cc][tb][:],
                        start=(cc == 0),
                        stop=(cc == NCC - 1),
                    )
                nc.scalar.activation(
                    kt[d][tb][:], ps[:], Ident, bias=bk_sb[:, d : d + 1], scale=1.0
                )
        for tb in range(NQB):
            for j in range(4):
                ti = tb * 4 + j
                ps = ps_qkv.tile([P, 512], f32)
                for cc in range(NCC):
                    nc.tensor.matmul(
                        ps[:],
                        lhsT=xts[cc][tb][:, j * P : (j + 1) * P],
                        rhs=wv[cc][:],
                        start=(cc == 0),
                        stop=(cc == NCC - 1),
                    )
                nc.scalar.activation(v_s[ti][:], ps[:], Copy)

        # ---- phase 2: S_T rows (s on partitions, q on free axis) ----
        for si in range(NT):
            qb0 = si // 4
            for qb in range(qb0, NQB):
                ps = ps_st.tile([P, 512], f32)
                for d in range(ND):
                    nc.tensor.matmul(
                        ps[:],
                        lhsT=kt[d][si // 4][:, (si % 4) * P : (si % 4 + 1) * P],
                        rhs=qt[d][qb][:],
                        start=(d == 0),
                        stop=(d == ND - 1),
                    )
                nc.scalar.activation(
                    E[si][:, qb * 512 : (qb + 1) * 512], ps[:], Exp, scale=float(SCALE)
                )
            # zero the strictly-lower-triangular part of the diagonal 128x128
            # window: keep element iff q >= s  <=>  col - row >= 0
            nc.gpsimd.affine_select(
                out=E[si][:, si * P : (si + 1) * P],
                in_=E[si][:, si * P : (si + 1) * P],
                pattern=[[1, P]],
                compare_op=mybir.AluOpType.is_ge,
                fill=0.0,
                base=0,
                channel_multiplier=-1,
            )
            # softmax-over-q denominator for these 128 s rows (valid q range
            # starts at the diagonal)
            nc.vector.reduce_sum(
                den[si][:], E[si][:, si * P : T], axis=mybir.AxisListType.X
            )
            nc.vector.reciprocal(rden[si][:], den[si][:])
            # fold bias + 1/denom into the v rows (normalization is over the
            # contraction axis of the attn matmul, so it must ride on v)
            nc.vector.tensor_add(v_s[si][:], v_s[si][:], bv_sb[:])
            nc.vector.tensor_scalar_mul(v_s[si][:], v_s[si][:], rden[si][:])

        # ---- phase 3: attn[q] = sum_s E[s, q] * v_scaled[s] ----
        for qi in range(NT):
            ps = ps_at.tile([P, 512], f32)
            for si in range(qi + 1):
                nc.tensor.matmul(
                    ps[:],
                    lhsT=E[si][:, qi * P : (qi + 1) * P],
                    rhs=v_s[si][:],
                    start=(si == 0),
                    stop=(si == qi),
                )
            ao = ao_pool.tile([P, K], f32)
            nc.vector.tensor_copy(ao[:], ps[:])
            nc.sync.dma_start(attn_d[qi * P : (qi + 1) * P, :], ao[:])

    nc.compile()
    return nc


def _get_nc():
    if "nc" not in _CACHE:
        _CACHE["nc"] = _build_nc()
    return _CACHE["nc"]


def _make_in_maps(x, Wq, bq, Wk, bk, Wv, bv):
    wqT = np.ascontiguousarray(Wq.T).astype(np.float16)
    wkT = np.ascontiguousarray(Wk.T).astype(np.float16)
    wvT = np.ascontiguousarray(Wv.T).astype(np.float16)
    bq32 = np.ascontiguousarray(bq).astype(np.float32)
    bk32 = np.ascontiguousarray(bk).astype(np.float32)
    bv16 = np.ascontiguousarray(bv).astype(np.float16)
    in_maps = []
    for i in range(N_CORES):
        xT = np.ascontiguousarray(x[i].T).astype(np.float16)
        in_maps.append(
            {
                "xT": xT,
                "wqT": wqT,
                "wkT": wkT,
                "wvT": wvT,
                "bq": bq32,
                "bk": bk32,
                "bv": bv16,
            }
        )
    return in_maps


def kernel(x, Wq, bq, Wk, bk, Wv, bv):
    from concourse.bass_utils import run_bass_kernel_spmd

    x = np.asarray(x, dtype=np.float32)
    Wq, Wk, Wv = (np.asarray(w, dtype=np.float32) for w in (Wq, Wk, Wv))
    bq, bk, bv = (np.asarray(b, dtype=np.float32) for b in (bq, bk, bv))

    nc = _get_nc()
    in_maps = _make_in_maps(x, Wq, bq, Wk, bk, Wv, bv)
    res = run_bass_kernel_spmd(nc, in_maps, core_ids=list(range(N_CORES)))

    out = np.empty((B, T, C + K), dtype=np.float32)
    out[:, :, :C] = x
    for i in range(N_CORES):
        out[i, :, C:] = res.results[i]["attn"]
    return out


# revision 4
# speedup vs baseline: 2.1589x; 2.1589x over previous
"""AttentionBlock kernel for 8 Trainium2 NeuronCores.

Reference (per batch element b):
    q = x @ Wq.T + bq; k = x @ Wk.T + bk; v = x @ Wv.T + bv
    scores[q,s] = q . k, causal-masked (s <= q valid)
    probs = softmax(scores / sqrt(512), axis=QUERY)   # normalized over q!
    attn = probs @ v
    out = concat([x, attn], -1)

Sharding: data-parallel over batch B=8 -> one batch element per core,
weights replicated, no collectives.

Device algorithm (bq = bk = 0 per the problem spec, asserted on host):
    scores = x (Wq^T Wk) x^T, so instead of two projections we compute
    G = Wq^T Wk (tiny) and y^T = G^T x^T (one projection), then
    S_T = scores^T in [s, q] layout via x^T-slices against y^T. The
    query-axis softmax is then a free-axis reduction, S_T is directly
    the lhsT operand for attn = S_T.T @ v, and the 1/denom[s]
    normalization rides on the v rows (it varies along the contraction
    axis). Causality skips all fully-masked blocks and narrows the
    diagonal ones. Host pre-transposes x (and Wv) so every matmul
    contraction lands on the partition axis; everything streams in
    fp16 with fp32 PSUM accumulation.
"""

import numpy as np

B, T, C, K = 8, 2048, 512, 512
P = 128
NCC = C // P  # contraction chunks (4)
NT = T // P  # 16 row tiles
NQB = T // 512  # 4 query blocks of 512
SCALE = 1.0 / np.sqrt(512.0)
N_CORES = 8

_CACHE = {}


def _build_nc(repeat=1):
    from contextlib import ExitStack

    import concourse.bass as bass
    import concourse.tile as tile
    from concourse import bacc, mybir

    f16 = mybir.dt.float16
    f32 = mybir.dt.float32

    nc = bacc.Bacc("TRN2", target_bir_lowering=False, debug=False)

    xT_h = nc.dram_tensor("xT", [C, T], f16, kind="ExternalInput")
    wq_h = nc.dram_tensor("wq", [K, C], f16, kind="ExternalInput")
    wk_h = nc.dram_tensor("wk", [K, C], f16, kind="ExternalInput")
    wvT_h = nc.dram_tensor("wvT", [C, K], f16, kind="ExternalInput")
    bv_h = nc.dram_tensor("bv", [K], f16, kind="ExternalInput")
    attn_h = nc.dram_tensor("attn", [T, K], f32, kind="ExternalOutput")
    attn_d = attn_h.ap()

    with tile.TileContext(nc) as tc, ExitStack() as ctx:
        # repeat>1 wraps the whole body in a HW loop — used only by the
        # benchmark harness to measure per-execution device time
        if repeat > 1:
            ctx.enter_context(tc.For_i(0, repeat, 1))

        sb = ctx.enter_context(tc.tile_pool(name="sb", bufs=1))
        ao_pool = ctx.enter_context(tc.tile_pool(name="ao", bufs=4))
        ps_qkv = ctx.enter_context(tc.tile_pool(name="ps_qkv", bufs=2, space="PSUM"))
        ps_st = ctx.enter_context(tc.tile_pool(name="ps_st", bufs=4, space="PSUM"))
        ps_at = ctx.enter_context(tc.tile_pool(name="ps_at", bufs=2, space="PSUM"))

        Exp = mybir.ActivationFunctionType.Exp
        Copy = mybir.ActivationFunctionType.Copy

        # prime the ACT table set (exp_and_others also covers the copies) so
        # the one-time ~2.7us table load overlaps the input DMAs
        warm = sb.tile([P, 1], f32, tag="warm")
        nc.vector.memset(warm[:], 0.0)
        nc.scalar.activation(warm[:], warm[:], Exp)

        # ---- inputs to SBUF ----
        wq = [sb.tile([P, C], f16, name=f"wq{d}", tag=f"wq{d}") for d in range(NCC)]
        wk = [sb.tile([P, C], f16, name=f"wk{d}", tag=f"wk{d}") for d in range(NCC)]
        wv = [sb.tile([P, K], f16, name=f"wv{cc}", tag=f"wv{cc}") for cc in range(NCC)]
        for d in range(NCC):
            nc.sync.dma_start(wq[d][:], wq_h.ap()[d * P : (d + 1) * P, :])
            nc.sync.dma_start(wk[d][:], wk_h.ap()[d * P : (d + 1) * P, :])
            nc.sync.dma_start(wv[d][:], wvT_h.ap()[d * P : (d + 1) * P, :])
        bv_sb = sb.tile([P, K], f16, tag="bv_sb")
        nc.sync.dma_start(bv_sb[:], bass.AP(tensor=bv_h, offset=0, ap=[[0, P], [1, K]]))

        # x^T in [c, t] layout, 4 partition chunks x 4 column blocks
        xts = [
            [
                sb.tile([P, 512], f16, name=f"xts{cc}_{tb}", tag=f"xts{cc}_{tb}")
                for tb in range(NQB)
            ]
            for cc in range(NCC)
        ]
        for tb in range(NQB):
            for cc in range(NCC):
                nc.sync.dma_start(
                    xts[cc][tb][:],
                    xT_h.ap()[cc * P : (cc + 1) * P, tb * 512 : (tb + 1) * 512],
                )

        # ---- persistent intermediates ----
        g_sb = [sb.tile([P, C], f16, name=f"g{cc}", tag=f"g{cc}") for cc in range(NCC)]
        yt = [
            [sb.tile([P, 512], f16, name=f"yt{cc}_{qb}", tag=f"yt{cc}_{qb}") for qb in range(NQB)]
            for cc in range(NCC)
        ]
        v_s = [sb.tile([P, K], f16, name=f"v{ti}", tag=f"v{ti}") for ti in range(NT)]
        E = [sb.tile([P, T], f16, name=f"E{si}", tag=f"E{si}") for si in range(NT)]
        den = [sb.tile([P, 1], f32, name=f"den{si}", tag=f"den{si}") for si in range(NT)]
        rden = [sb.tile([P, 1], f32, name=f"rden{si}", tag=f"rden{si}") for si in range(NT)]

        # ---- phase 0: G = Wq^T Wk  (G[c1, c2], c1 on partitions) ----
        for c1 in range(NCC):
            ps = ps_qkv.tile([P, 512], f32)
            for d in range(NCC):
                nc.tensor.matmul(
                    ps[:],
                    lhsT=wq[d][:, c1 * P : (c1 + 1) * P],
                    rhs=wk[d][:],
                    start=(d == 0),
                    stop=(d == NCC - 1),
                )
            nc.scalar.activation(g_sb[c1][:], ps[:], Copy)

        # ---- phase 1a: y^T = G^T x^T in [c2, q] layout ----
        for c2 in range(NCC):
            for qb in range(NQB):
                ps = ps_qkv.tile([P, 512], f32)
                for c1 in range(NCC):
                    nc.tensor.matmul(
                        ps[:],
                        lhsT=g_sb[c1][:, c2 * P : (c2 + 1) * P],
                        rhs=xts[c1][qb][:],
                        start=(c1 == 0),
                        stop=(c1 == NCC - 1),
                    )
                nc.scalar.activation(yt[c2][qb][:], ps[:], Copy)

        # ---- phase 1b: v in [t, dv] layout ----
        for tb in range(NQB):
            for j in range(4):
                ti = tb * 4 + j
                ps = ps_qkv.tile([P, 512], f32)
                for cc in range(NCC):
                    nc.tensor.matmul(
                        ps[:],
                        lhsT=xts[cc][tb][:, j * P : (j + 1) * P],
                        rhs=wv[cc][:],
                        start=(cc == 0),
                        stop=(cc == NCC - 1),
                    )
                nc.scalar.activation(v_s[ti][:], ps[:], Copy)

        # ---- phase 2: S_T rows (s on partitions, q free), causal ----
        for si in range(NT):
            qb0 = si // 4
            off = (si % 4) * P  # diagonal offset inside block qb0
            for qb in range(qb0, NQB):
                lo = off if qb == qb0 else 0
                ps = ps_st.tile([P, 512], f32)
                for cc in range(NCC):
                    nc.tensor.matmul(
                        ps[:, lo:512],
                        lhsT=xts[cc][si // 4][:, off : off + P],
                        rhs=yt[cc][qb][:, lo:512],
                        start=(cc == 0),
                        stop=(cc == NCC - 1),
                    )
                nc.scalar.activation(
                    E[si][:, qb * 512 + lo : (qb + 1) * 512],
                    ps[:, lo:512],
                    Exp,
                    scale=float(SCALE),
                )
            # zero the strictly-lower-triangular part of the diagonal 128x128
            # window: keep element iff q >= s  <=>  col - row >= 0
            nc.gpsimd.affine_select(
                out=E[si][:, si * P : (si + 1) * P],
                in_=E[si][:, si * P : (si + 1) * P],
                pattern=[[1, P]],
                compare_op=mybir.AluOpType.is_ge,
                fill=0.0,
                base=0,
                channel_multiplier=-1,
            )
            # softmax-over-q denominator for these 128 s rows (valid q range
            # starts at the diagonal)
            nc.vector.reduce_sum(
                den[si][:], E[si][:, si * P : T], axis=mybir.AxisListType.X
            )
            nc.vector.reciprocal(rden[si][:], den[si][:])
            # fold bias + 1/denom into the v rows (normalization is over the
            # contraction axis of the attn matmul, so it must ride on v)
            nc.vector.tensor_add(v_s[si][:], v_s[si][:], bv_sb[:])
            nc.vector.tensor_scalar_mul(v_s[si][:], v_s[si][:], rden[si][:])

        # ---- phase 3: attn[q] = sum_s E[s, q-window] * v_scaled[s] ----
        for qi in range(NT):
            ps = ps_at.tile([P, 512], f32)
            for si in range(qi + 1):
                nc.tensor.matmul(
                    ps[:],
                    lhsT=E[si][:, qi * P : (qi + 1) * P],
                    rhs=v_s[si][:],
                    start=(si == 0),
                    stop=(si == qi),
                )
            ao = ao_pool.tile([P, K], f32)
            nc.vector.tensor_copy(ao[:], ps[:])
            nc.sync.dma_start(attn_d[qi * P : (qi + 1) * P, :], ao[:])

    nc.compile()
    return nc


def _get_nc(repeat=1):
    key = ("nc", repeat)
    if key not in _CACHE:
        _CACHE[key] = _build_nc(repeat)
    return _CACHE[key]


def _make_in_maps(x, Wq, bq, Wk, bk, Wv, bv):
    assert np.all(bq == 0.0) and np.all(bk == 0.0), (
        "kernel folds Wq^T Wk; nonzero q/k biases unsupported"
    )
    wq16 = np.ascontiguousarray(Wq).astype(np.float16)
    wk16 = np.ascontiguousarray(Wk).astype(np.float16)
    wvT = np.ascontiguousarray(Wv.T).astype(np.float16)
    bv16 = np.ascontiguousarray(bv).astype(np.float16)
    in_maps = []
    for i in range(N_CORES):
        xT = np.ascontiguousarray(x[i].T).astype(np.float16)
        in_maps.append(
            {"xT": xT, "wq": wq16, "wk": wk16, "wvT": wvT, "bv": bv16}
        )
    return in_maps


def kernel(x, Wq, bq, Wk, bk, Wv, bv):
    from concourse.bass_utils import run_bass_kernel_spmd

    x = np.asarray(x, dtype=np.float32)
    Wq, Wk, Wv = (np.asarray(w, dtype=np.float32) for w in (Wq, Wk, Wv))
    bq, bk, bv = (np.asarray(b, dtype=np.float32) for b in (bq, bk, bv))

    nc = _get_nc()
    in_maps = _make_in_maps(x, Wq, bq, Wk, bk, Wv, bv)
    res = run_bass_kernel_spmd(nc, in_maps, core_ids=list(range(N_CORES)))

    out = np.empty((B, T, C + K), dtype=np.float32)
    out[:, :, :C] = x
    for i in range(N_CORES):
        out[i, :, C:] = res.results[i]["attn"]
    return out


# revision 6
# speedup vs baseline: 14.3357x; 6.6402x over previous
"""AttentionBlock kernel for 8 Trainium2 NeuronCores.

Reference (per batch element b):
    q = x @ Wq.T + bq; k = x @ Wk.T + bk; v = x @ Wv.T + bv
    scores[q,s] = q . k, causal-masked (s <= q valid)
    probs = softmax(scores / sqrt(512), axis=QUERY)   # normalized over q!
    attn = probs @ v
    out = concat([x, attn], -1)

Sharding: data-parallel over batch B=8 -> one batch element per core,
weights replicated, no collectives.

Device algorithm (bq = bk = 0 per the problem spec, asserted on host):
    scores = x (Wq^T Wk) x^T, so instead of two projections we compute
    G = Wq^T Wk (tiny) and y^T = G^T x^T (one projection), then
    S_T = scores^T in [s, q] layout via x^T-slices against y^T. The
    query-axis softmax is then a free-axis reduction, S_T is directly
    the lhsT operand for attn = S_T.T @ v, and the 1/denom[s]
    normalization rides on the v rows (it varies along the contraction
    axis). Causality skips all fully-masked blocks and narrows the
    diagonal ones. Host pre-transposes x (and Wv) so every matmul
    contraction lands on the partition axis; everything streams in
    fp16 with fp32 PSUM accumulation. The y^T and S_T inner loops keep
    the stationary operand fixed across the moving blocks (groups
    interleaved over PSUM banks) to amortize weight loads.
"""

import numpy as np

B, T, C, K = 8, 2048, 512, 512
P = 128
NCC = C // P  # contraction chunks (4)
NT = T // P  # 16 row tiles
NQB = T // 512  # 4 query blocks of 512
SCALE = 1.0 / np.sqrt(512.0)
N_CORES = 8

_CACHE = {}


def _build_nc(repeat=1):
    from contextlib import ExitStack

    import concourse.bass as bass
    import concourse.tile as tile
    from concourse import bacc, mybir

    f16 = mybir.dt.float16
    f32 = mybir.dt.float32

    nc = bacc.Bacc("TRN2", target_bir_lowering=False, debug=False)

    xT_h = nc.dram_tensor("xT", [C, T], f16, kind="ExternalInput")
    wq_h = nc.dram_tensor("wq", [K, C], f16, kind="ExternalInput")
    wk_h = nc.dram_tensor("wk", [K, C], f16, kind="ExternalInput")
    wvT_h = nc.dram_tensor("wvT", [C, K], f16, kind="ExternalInput")
    bv_h = nc.dram_tensor("bv", [K], f16, kind="ExternalInput")
    attn_h = nc.dram_tensor("attn", [T, K], f32, kind="ExternalOutput")
    attn_d = attn_h.ap()

    # dram view [p, chunk, col] of a row-major [512, ncol] weight
    def chunked(h, ncol):
        return bass.AP(tensor=h, offset=0, ap=[[ncol, P], [P * ncol, NCC], [1, ncol]])

    with tile.TileContext(nc) as tc, ExitStack() as ctx:
        # repeat>1 wraps the whole body in a HW loop — used only by the
        # benchmark harness to measure per-execution device time
        if repeat > 1:
            ctx.enter_context(tc.For_i(0, repeat, 1))

        sb = ctx.enter_context(tc.tile_pool(name="sb", bufs=1))
        ao_pool = ctx.enter_context(tc.tile_pool(name="ao", bufs=2))
        ps = ctx.enter_context(tc.tile_pool(name="ps", bufs=4, space="PSUM"))
        ps_at = ctx.enter_context(tc.tile_pool(name="ps_at", bufs=2, space="PSUM"))

        Exp = mybir.ActivationFunctionType.Exp
        Copy = mybir.ActivationFunctionType.Copy

        # prime the ACT table set (exp_and_others also covers the copies) so
        # the one-time ~2.7us table load overlaps the input DMAs
        warm = sb.tile([P, 1], f32, tag="warm")
        nc.vector.memset(warm[:], 0.0)
        nc.scalar.activation(warm[:], warm[:], Exp)

        # ---- inputs to SBUF (one DMA per tensor / x chunk) ----
        wq_t = sb.tile([P, NCC, C], f16, tag="wq_t")
        wk_t = sb.tile([P, NCC, C], f16, tag="wk_t")
        wv_t = sb.tile([P, NCC, K], f16, tag="wv_t")
        nc.sync.dma_start(wq_t[:], chunked(wq_h, C))
        nc.sync.dma_start(wk_t[:], chunked(wk_h, C))
        nc.sync.dma_start(wv_t[:], chunked(wvT_h, K))
        bv_sb = sb.tile([P, K], f16, tag="bv_sb")
        nc.sync.dma_start(bv_sb[:], bass.AP(tensor=bv_h, offset=0, ap=[[0, P], [1, K]]))

        # x^T in [c, t] layout, 4 partition chunks
        xts = [
            sb.tile([P, T], f16, name=f"xts{cc}", tag=f"xts{cc}") for cc in range(NCC)
        ]
        for cc in range(NCC):
            nc.sync.dma_start(xts[cc][:], xT_h.ap()[cc * P : (cc + 1) * P, :])

        # ---- persistent intermediates ----
        g_sb = [sb.tile([P, C], f16, name=f"g{cc}", tag=f"g{cc}") for cc in range(NCC)]
        yt = [
            [sb.tile([P, 512], f16, name=f"yt{cc}_{qb}", tag=f"yt{cc}_{qb}") for qb in range(NQB)]
            for cc in range(NCC)
        ]
        v_s = [sb.tile([P, K], f16, name=f"v{ti}", tag=f"v{ti}") for ti in range(NT)]
        E = [sb.tile([P, T], f16, name=f"E{si}", tag=f"E{si}") for si in range(NT)]
        den = [sb.tile([P, 1], f32, name=f"den{si}", tag=f"den{si}") for si in range(NT)]
        rden = [sb.tile([P, 1], f32, name=f"rden{si}", tag=f"rden{si}") for si in range(NT)]

        # ---- phase 0: G = Wq^T Wk  (G[c1, c2], c1 on partitions) ----
        for c1 in range(NCC):
            pg = ps.tile([P, 512], f32, tag="mm")
            for d in range(NCC):
                nc.tensor.matmul(
                    pg[:],
                    lhsT=wq_t[:, d, c1 * P : (c1 + 1) * P],
                    rhs=wk_t[:, d, :],
                    start=(d == 0),
                    stop=(d == NCC - 1),
                )
            nc.scalar.activation(g_sb[c1][:], pg[:], Copy)

        # ---- phase 1a: y^T = G^T x^T in [c2, q] layout ----
        # stationary operand fixed across the qb loop; the 4 accumulation
        # groups interleave over 4 PSUM banks
        for c2 in range(NCC):
            py = [ps.tile([P, 512], f32, name=f"py{qb}", tag="mm") for qb in range(NQB)]
            for c1 in range(NCC):
                for qb in range(NQB):
                    nc.tensor.matmul(
                        py[qb][:],
                        lhsT=g_sb[c1][:, c2 * P : (c2 + 1) * P],
                        rhs=xts[c1][:, qb * 512 : (qb + 1) * 512],
                        start=(c1 == 0),
                        stop=(c1 == NCC - 1),
                    )
            for qb in range(NQB):
                nc.scalar.activation(yt[c2][qb][:], py[qb][:], Copy)

        # ---- phase 1b: v in [t, dv] layout ----
        for ti in range(NT):
            tb, j = divmod(ti, 4)
            pv = ps.tile([P, 512], f32, tag="mm")
            for cc in range(NCC):
                nc.tensor.matmul(
                    pv[:],
                    lhsT=xts[cc][:, ti * P : (ti + 1) * P],
                    rhs=wv_t[:, cc, :],
                    start=(cc == 0),
                    stop=(cc == NCC - 1),
                )
            nc.scalar.activation(v_s[ti][:], pv[:], Copy)

        # ---- phase 2: S_T rows (s on partitions, q free), causal ----
        for si in range(NT):
            qb0 = si // 4
            off = (si % 4) * P  # diagonal offset inside block qb0
            pss = {qb: ps.tile([P, 512], f32, name=f"pss{qb}", tag="mm") for qb in range(qb0, NQB)}
            for cc in range(NCC):
                for qb in range(qb0, NQB):
                    lo = off if qb == qb0 else 0
                    nc.tensor.matmul(
                        pss[qb][:, lo:512],
                        lhsT=xts[cc][:, si * P : (si + 1) * P],
                        rhs=yt[cc][qb][:, lo:512],
                        start=(cc == 0),
                        stop=(cc == NCC - 1),
                    )
            for qb in range(qb0, NQB):
                lo = off if qb == qb0 else 0
                nc.scalar.activation(
                    E[si][:, qb * 512 + lo : (qb + 1) * 512],
                    pss[qb][:, lo:512],
                    Exp,
                    scale=float(SCALE),
                )
            # zero the strictly-lower-triangular part of the diagonal 128x128
            # window: keep element iff q >= s  <=>  col - row >= 0
            nc.gpsimd.affine_select(
                out=E[si][:, si * P : (si + 1) * P],
                in_=E[si][:, si * P : (si + 1) * P],
                pattern=[[1, P]],
                compare_op=mybir.AluOpType.is_ge,
                fill=0.0,
                base=0,
                channel_multiplier=-1,
            )
            # softmax-over-q denominator for these 128 s rows (valid q range
            # starts at the diagonal)
            nc.vector.reduce_sum(
                den[si][:], E[si][:, si * P : T], axis=mybir.AxisListType.X
            )
            nc.vector.reciprocal(rden[si][:], den[si][:])
            # fold bias + 1/denom into the v rows (normalization is over the
            # contraction axis of the attn matmul, so it must ride on v)
            nc.vector.tensor_add(v_s[si][:], v_s[si][:], bv_sb[:])
            nc.vector.tensor_scalar_mul(v_s[si][:], v_s[si][:], rden[si][:])

        # ---- phase 3: attn[q] = sum_s E[s, q-window] * v_scaled[s] ----
        # output staged in groups of 4 q-tiles -> 4 big DMAs
        for g in range(4):
            ao = ao_pool.tile([P, 4, 512], f32, tag="ao")
            for j in range(4):
                qi = g * 4 + j
                pa = ps_at.tile([P, 512], f32, tag="at")
                for si in range(qi + 1):
                    nc.tensor.matmul(
                        pa[:],
                        lhsT=E[si][:, qi * P : (qi + 1) * P],
                        rhs=v_s[si][:],
                        start=(si == 0),
                        stop=(si == qi),
                    )
                nc.vector.tensor_copy(ao[:, j, :], pa[:])
            nc.sync.dma_start(
                bass.AP(
                    tensor=attn_h,
                    offset=g * 512 * 512,
                    ap=[[512, P], [P * 512, 4], [1, 512]],
                ),
                ao[:],
            )

    nc.compile()
    return nc


def _get_nc(repeat=1):
    key = ("nc", repeat)
    if key not in _CACHE:
        _CACHE[key] = _build_nc(repeat)
    return _CACHE[key]


def _make_in_maps(x, Wq, bq, Wk, bk, Wv, bv):
    assert np.all(bq == 0.0) and np.all(bk == 0.0), (
        "kernel folds Wq^T Wk; nonzero q/k biases unsupported"
    )
    wq16 = np.ascontiguousarray(Wq).astype(np.float16)
    wk16 = np.ascontiguousarray(Wk).astype(np.float16)
    wvT = np.ascontiguousarray(Wv.T).astype(np.float16)
    bv16 = np.ascontiguousarray(bv).astype(np.float16)
    in_maps = []
    for i in range(N_CORES):
        xT = np.ascontiguousarray(x[i].T).astype(np.float16)
        in_maps.append(
            {"xT": xT, "wq": wq16, "wk": wk16, "wvT": wvT, "bv": bv16}
        )
    return in_maps


def kernel(x, Wq, bq, Wk, bk, Wv, bv):
    from concourse.bass_utils import run_bass_kernel_spmd

    x = np.asarray(x, dtype=np.float32)
    Wq, Wk, Wv = (np.asarray(w, dtype=np.float32) for w in (Wq, Wk, Wv))
    bq, bk, bv = (np.asarray(b, dtype=np.float32) for b in (bq, bk, bv))

    nc = _get_nc()
    in_maps = _make_in_maps(x, Wq, bq, Wk, bk, Wv, bv)
    res = run_bass_kernel_spmd(nc, in_maps, core_ids=list(range(N_CORES)))

    out = np.empty((B, T, C + K), dtype=np.float32)
    out[:, :, :C] = x
    for i in range(N_CORES):
        out[i, :, C:] = res.results[i]["attn"]
    return out


# revision 7
# speedup vs baseline: 29.3510x; 2.0474x over previous
"""AttentionBlock kernel for 8 Trainium2 NeuronCores.

Reference (per batch element b):
    q = x @ Wq.T + bq; k = x @ Wk.T + bk; v = x @ Wv.T + bv
    scores[q,s] = q . k, causal-masked (s <= q valid)
    probs = softmax(scores / sqrt(512), axis=QUERY)   # normalized over q!
    attn = probs @ v
    out = concat([x, attn], -1)

Sharding: data-parallel over batch B=8 -> one batch element per core,
weights replicated, no collectives.

Device algorithm (bq = bk = 0 per the problem spec, asserted on host):
    scores = x (Wq^T Wk) x^T, so instead of two projections we compute
    G = Wq^T Wk (tiny) and y^T = G^T x^T (one projection), then
    S_T = scores^T in [s, q] layout via x^T-slices against y^T. The
    query-axis softmax is then a free-axis reduction, S_T is directly
    the lhsT operand for attn = S_T.T @ v, and the 1/denom[s]
    normalization rides on the v rows (it varies along the contraction
    axis). Causality skips all fully-masked blocks and narrows the
    diagonal ones. Host pre-transposes x (and Wv) so every matmul
    contraction lands on the partition axis; everything streams in
    fp16 with fp32 PSUM accumulation. The y^T and S_T inner loops keep
    the stationary operand fixed across the moving blocks (groups
    interleaved over PSUM banks) to amortize weight loads.
"""

import numpy as np

B, T, C, K = 8, 2048, 512, 512
P = 128
NCC = C // P  # contraction chunks (4)
NT = T // P  # 16 row tiles
NQB = T // 512  # 4 query blocks of 512
SCALE = 1.0 / np.sqrt(512.0)
N_CORES = 8

_CACHE = {}


def _build_nc(repeat=1):
    from contextlib import ExitStack

    import concourse.bass as bass
    import concourse.tile as tile
    from concourse import bacc, mybir

    f16 = mybir.dt.float16
    f32 = mybir.dt.float32

    nc = bacc.Bacc("TRN2", target_bir_lowering=False, debug=False)

    xT_h = nc.dram_tensor("xT", [C, T], f16, kind="ExternalInput")
    wq_h = nc.dram_tensor("wq", [K, C], f16, kind="ExternalInput")
    wk_h = nc.dram_tensor("wk", [K, C], f16, kind="ExternalInput")
    wvT_h = nc.dram_tensor("wvT", [C, K], f16, kind="ExternalInput")
    bv_h = nc.dram_tensor("bv", [K], f16, kind="ExternalInput")
    attn_h = nc.dram_tensor("attn", [T, K], f32, kind="ExternalOutput")
    attn_d = attn_h.ap()

    # dram view [p, chunk, col] of a row-major [512, ncol] weight
    def chunked(h, ncol):
        return bass.AP(tensor=h, offset=0, ap=[[ncol, P], [P * ncol, NCC], [1, ncol]])

    with tile.TileContext(nc) as tc, ExitStack() as ctx:
        # repeat>1 wraps the whole body in a HW loop — used only by the
        # benchmark harness to measure per-execution device time
        if repeat > 1:
            ctx.enter_context(tc.For_i(0, repeat, 1))

        sb = ctx.enter_context(tc.tile_pool(name="sb", bufs=1))
        ao_pool = ctx.enter_context(tc.tile_pool(name="ao", bufs=2))
        ps = ctx.enter_context(tc.tile_pool(name="ps", bufs=6, space="PSUM"))
        ps_at = ctx.enter_context(tc.tile_pool(name="ps_at", bufs=2, space="PSUM"))

        Exp = mybir.ActivationFunctionType.Exp
        Copy = mybir.ActivationFunctionType.Copy

        # prime the ACT table set (exp_and_others also covers the copies) so
        # the one-time ~2.7us table load overlaps the input DMAs
        warm = sb.tile([P, 1], f32, tag="warm")
        nc.vector.memset(warm[:], 0.0)
        nc.scalar.activation(warm[:], warm[:], Exp)

        # ---- inputs to SBUF (one DMA per tensor / x chunk) ----
        wq_t = sb.tile([P, NCC, C], f16, tag="wq_t")
        wk_t = sb.tile([P, NCC, C], f16, tag="wk_t")
        wv_t = sb.tile([P, NCC, K], f16, tag="wv_t")
        nc.sync.dma_start(wq_t[:], chunked(wq_h, C))
        nc.sync.dma_start(wk_t[:], chunked(wk_h, C))
        nc.sync.dma_start(wv_t[:], chunked(wvT_h, K))
        bv_sb = sb.tile([P, K], f16, tag="bv_sb")
        nc.sync.dma_start(bv_sb[:], bass.AP(tensor=bv_h, offset=0, ap=[[0, P], [1, K]]))

        # x^T in [c, t] layout, 4 partition chunks
        xts = [
            sb.tile([P, T], f16, name=f"xts{cc}", tag=f"xts{cc}") for cc in range(NCC)
        ]
        for cc in range(NCC):
            nc.sync.dma_start(xts[cc][:], xT_h.ap()[cc * P : (cc + 1) * P, :])

        # ---- persistent intermediates ----
        g_sb = [sb.tile([P, C], f16, name=f"g{cc}", tag=f"g{cc}") for cc in range(NCC)]
        yt = [
            [sb.tile([P, 512], f16, name=f"yt{cc}_{qb}", tag=f"yt{cc}_{qb}") for qb in range(NQB)]
            for cc in range(NCC)
        ]
        v_s = [sb.tile([P, K], f16, name=f"v{ti}", tag=f"v{ti}") for ti in range(NT)]
        E = [sb.tile([P, T], f16, name=f"E{si}", tag=f"E{si}") for si in range(NT)]
        den = [sb.tile([P, 1], f32, name=f"den{si}", tag=f"den{si}") for si in range(NT)]
        rden = [sb.tile([P, 1], f32, name=f"rden{si}", tag=f"rden{si}") for si in range(NT)]

        # ---- phase 0: G = Wq^T Wk  (G[c1, c2], c1 on partitions) ----
        for c1 in range(NCC):
            pg = ps.tile([P, 512], f32, tag="mm")
            for d in range(NCC):
                nc.tensor.matmul(
                    pg[:],
                    lhsT=wq_t[:, d, c1 * P : (c1 + 1) * P],
                    rhs=wk_t[:, d, :],
                    start=(d == 0),
                    stop=(d == NCC - 1),
                )
            nc.scalar.activation(g_sb[c1][:], pg[:], Copy)

        # ---- phase 1a: y^T = G^T x^T in [c2, q] layout ----
        for c2 in range(NCC):
            for qb in range(NQB):
                py = ps.tile([P, 512], f32, tag="mm")
                for c1 in range(NCC):
                    nc.tensor.matmul(
                        py[:],
                        lhsT=g_sb[c1][:, c2 * P : (c2 + 1) * P],
                        rhs=xts[c1][:, qb * 512 : (qb + 1) * 512],
                        start=(c1 == 0),
                        stop=(c1 == NCC - 1),
                    )
                nc.scalar.activation(yt[c2][qb][:], py[:], Copy)

        # ---- phase 1b: v in [t, dv] layout ----
        for ti in range(NT):
            tb, j = divmod(ti, 4)
            pv = ps.tile([P, 512], f32, tag="mm")
            for cc in range(NCC):
                nc.tensor.matmul(
                    pv[:],
                    lhsT=xts[cc][:, ti * P : (ti + 1) * P],
                    rhs=wv_t[:, cc, :],
                    start=(cc == 0),
                    stop=(cc == NCC - 1),
                )
            nc.scalar.activation(v_s[ti][:], pv[:], Copy)

        # ---- phase 2: S_T rows (s on partitions, q free), causal ----
        for si in range(NT):
            qb0 = si // 4
            off = (si % 4) * P  # diagonal offset inside block qb0
            for qb in range(qb0, NQB):
                lo = off if qb == qb0 else 0
                pst = ps.tile([P, 512], f32, tag="mm")
                for cc in range(NCC):
                    nc.tensor.matmul(
                        pst[:, lo:512],
                        lhsT=xts[cc][:, si * P : (si + 1) * P],
                        rhs=yt[cc][qb][:, lo:512],
                        start=(cc == 0),
                        stop=(cc == NCC - 1),
                    )
                nc.scalar.activation(
                    E[si][:, qb * 512 + lo : (qb + 1) * 512],
                    pst[:, lo:512],
                    Exp,
                    scale=float(SCALE),
                )
            # zero the strictly-lower-triangular part of the diagonal 128x128
            # window: keep element iff q >= s  <=>  col - row >= 0
            nc.gpsimd.affine_select(
                out=E[si][:, si * P : (si + 1) * P],
                in_=E[si][:, si * P : (si + 1) * P],
                pattern=[[1, P]],
                compare_op=mybir.AluOpType.is_ge,
                fill=0.0,
                base=0,
                channel_multiplier=-1,
            )
            # softmax-over-q denominator for these 128 s rows (valid q range
            # starts at the diagonal)
            nc.vector.reduce_sum(
                den[si][:], E[si][:, si * P : T], axis=mybir.AxisListType.X
            )
            nc.vector.reciprocal(rden[si][:], den[si][:])
            # fold bias + 1/denom into the v rows (normalization is over the
            # contraction axis of the attn matmul, so it must ride on v)
            nc.vector.tensor_add(v_s[si][:], v_s[si][:], bv_sb[:])
            nc.vector.tensor_scalar_mul(v_s[si][:], v_s[si][:], rden[si][:])

        # ---- phase 3: attn[q] = sum_s E[s, q-window] * v_scaled[s] ----
        # output staged in groups of 4 q-tiles -> 4 big DMAs
        for g in range(4):
            ao = ao_pool.tile([P, 4, 512], f32, tag="ao")
            for j in range(4):
                qi = g * 4 + j
                pa = ps_at.tile([P, 512], f32, tag="at")
                for si in range(qi + 1):
                    nc.tensor.matmul(
                        pa[:],
                        lhsT=E[si][:, qi * P : (qi + 1) * P],
                        rhs=v_s[si][:],
                        start=(si == 0),
                        stop=(si == qi),
                    )
                nc.vector.tensor_copy(ao[:, j, :], pa[:])
            nc.sync.dma_start(
                bass.AP(
                    tensor=attn_h,
                    offset=g * 512 * 512,
                    ap=[[512, P], [P * 512, 4], [1, 512]],
                ),
                ao[:],
            )

    nc.compile()
    return nc


def _get_nc(repeat=1):
    key = ("nc", repeat)
    if key not in _CACHE:
        _CACHE[key] = _build_nc(repeat)
    return _CACHE[key]


def _make_in_maps(x, Wq, bq, Wk, bk, Wv, bv):
    assert np.all(bq == 0.0) and np.all(bk == 0.0), (
        "kernel folds Wq^T Wk; nonzero q/k biases unsupported"
    )
    wq16 = np.ascontiguousarray(Wq).astype(np.float16)
    wk16 = np.ascontiguousarray(Wk).astype(np.float16)
    wvT = np.ascontiguousarray(Wv.T).astype(np.float16)
    bv16 = np.ascontiguousarray(bv).astype(np.float16)
    in_maps = []
    for i in range(N_CORES):
        xT = np.ascontiguousarray(x[i].T).astype(np.float16)
        in_maps.append(
            {"xT": xT, "wq": wq16, "wk": wk16, "wvT": wvT, "bv": bv16}
        )
    return in_maps


def kernel(x, Wq, bq, Wk, bk, Wv, bv):
    from concourse.bass_utils import run_bass_kernel_spmd

    x = np.asarray(x, dtype=np.float32)
    Wq, Wk, Wv = (np.asarray(w, dtype=np.float32) for w in (Wq, Wk, Wv))
    bq, bk, bv = (np.asarray(b, dtype=np.float32) for b in (bq, bk, bv))

    nc = _get_nc()
    in_maps = _make_in_maps(x, Wq, bq, Wk, bk, Wv, bv)
    res = run_bass_kernel_spmd(nc, in_maps, core_ids=list(range(N_CORES)))

    out = np.empty((B, T, C + K), dtype=np.float32)
    out[:, :, :C] = x
    for i in range(N_CORES):
        out[i, :, C:] = res.results[i]["attn"]
    return out
